# revision 34
# baseline (speedup 1.0000x reference)
"""Multi-head attention Trainium2 Bass kernel, 8-way sharded.

Problem: x:[4,2048,512] fp32, Wq/Wk/Wv:[8,512,64] fp32 ->
         softmax(x@Wq_h @ (x@Wk_h)^T / sqrt(64)) @ (x@Wv_h), heads concat
         -> [4,2048,512] fp32.

Sharding: 8 cores = 4 batches x 2 head-groups (4 heads each). Each core
computes out[b, :, hg*256:(hg+1)*256]; the host gathers slices (no
collectives needed).

Per-core dataflow (fp16 operands; fp8 was tried and is numerically too
lossy for the 2e-2 gate -- peaked softmax rows amplify q/k quantization):
  - host supplies x[b].T as [128,4,2048] fp16 (d on partitions) and
    fp16 weights with head-pair column blocks
  - projections: per d-chunk accumulating matmuls; qT/kT stored with
    head pair blocks (heads 2b,2b+1 on partitions 0:64/64:128)
  - only ACT and DVE can read PSUM (BIR rule), so the softmax exp of
    every scores tile is split between them for throughput:
    ACT: exact exp (scale fused) -> fp16
    DVE: Schraudolph bit-trick exp (t = a*s+b -> int16 -> bitcast fp16)
    exp(s' - 2) bias is softmax-invariant, cancelling in the normalize;
    strict A/D alternation keeps tiles 3 apart (the PSUM-ring recycle
    distance) on opposite engines so ring latency hides
  - flipped AV: acc[q, dh] += ex_chunk.T @ vaug accumulates [128,65]
    per q-chunk in PSUM; vaug carries a 1.0 column so the same matmul
    accumulates the softmax denominator
  - tail per unit: one strided copy acc->SBUF (ACT), DVE reciprocal of
    the 8 denominators, then GPSIMD (SBUF-only) does the 8 normalize
    multiplies into the staging buffer
  - input DMAs spread across sync/scalar/gpsimd queues; output leaves
    in row-chunks as soon as all four heads of a q-half are done
"""

import numpy as np

import bass_rust as _bass_rust
import concourse.bass as bass
import concourse.tile as tile
from concourse import mybir
from concourse.bass_utils import run_bass_kernel_spmd

B, S, D, H, DH = 4, 2048, 512, 8, 64
P = 128
HL = H // 2          # heads per core
ND = D // P          # d chunks
NKC = S // P         # k chunks
NQH = 2              # q halves
QH = S // 2
NQC = QH // P        # q chunks per half
CDH = HL * DH        # per-core output columns
NJ = S // P          # output row chunks

LOG2E = 1.4426950408889634
CBIAS = 2.0                    # exp(s' - CBIAS); cancels in softmax
SCALE = DH ** -0.5
A16 = 1024.0 * LOG2E * SCALE
B16 = (15.0 - CBIAS * LOG2E) * 1024.0 - 46.0

F16 = mybir.dt.float16
F32 = mybir.dt.float32
I16 = mybir.dt.int16
EXP = mybir.ActivationFunctionType.Exp
COPY = mybir.ActivationFunctionType.Copy
ALU = mybir.AluOpType

_CACHE = {}


def _build():
    nc = bass.Bass()
    xt = nc.dram_tensor("xt", [P, ND, S], F16, kind="ExternalInput")
    # wq/wk: [d-part, d-chunk, head-pair block, 128 cols (2 heads x 64dh)]
    wq = nc.dram_tensor("wq", [P, ND, 2, P], F16, kind="ExternalInput")
    wk = nc.dram_tensor("wk", [P, ND, 2, P], F16, kind="ExternalInput")
    wv = nc.dram_tensor("wv", [P, ND, CDH], F16, kind="ExternalInput")
    out = nc.dram_tensor("out", [S, CDH], F32, kind="ExternalOutput")

    with tile.TileContext(nc) as tc:
        with tc.tile_pool(name="persist", bufs=1) as pers:
            xt_s = pers.tile([P, ND, S], F16)
            wq_s = pers.tile([P, ND, 2, P], F16)
            nc.gpsimd.dma_start(out=wq_s, in_=wq[:, :, :, :])
            wk_s = pers.tile([P, ND, 2, P], F16)
            nc.scalar.dma_start(out=wk_s, in_=wk[:, :, :, :])
            wv_s = pers.tile([P, ND, CDH], F16)
            # S-half pieces: all half-0 pieces land first (gate k00/q00)
            h0q = [nc.sync, nc.gpsimd, nc.scalar, nc.sync]
            for c in range(ND):
                h0q[c].dma_start(out=xt_s[:, c, 0:QH], in_=xt[:, c, 0:QH])
            nc.sync.dma_start(out=wv_s, in_=wv[:, :, :])
            h1q = [nc.gpsimd, nc.scalar, nc.sync, nc.gpsimd]
            for c in range(ND):
                h1q[c].dma_start(out=xt_s[:, c, QH:S], in_=xt[:, c, QH:S])

            # head pair hl//2; head hl%2 on partitions 64*(hl%2)+0:64
            qT = pers.tile([P, 2, NQH, QH], F16)
            kT = pers.tile([P, 2, S], F16)
            # V + 1.0 column: [k-part, kc, head, dh+1]
            vaug = pers.tile([P, NKC, HL, DH + 1], F16)
            outb = pers.tile([P, NJ, CDH], F32)
            bias = pers.tile([P, 1], F32)
            nc.vector.memset(bias, -CBIAS)
            nc.vector.memset(vaug[:, :, :, DH:DH + 1], 1.0)
            # load the Exp table during the input DMAs
            warm = pers.tile([1, 1], F32)
            nc.vector.memset(warm, 0.0)
            nc.scalar.activation(out=warm, in_=warm, func=EXP)

            # ---------------- phase emitters ----------------
            def _copy(eng, dst, src):
                if eng == 'A':
                    nc.scalar.activation(out=dst, in_=src, func=COPY)
                else:
                    nc.vector.tensor_copy(dst, src)

            def emit_qk_proj(pool, eng, w_s, blk, half, dst):
                # ps[2x64 dh-cols, q] for one head-pair block
                ps = pool.tile([P, QH], F32, tag="sc", name=f"pj{blk}{half}")
                for c in range(ND):
                    for n in range(2):
                        nc.tensor.matmul(
                            ps[:, n * 512:(n + 1) * 512],
                            lhsT=w_s[:, c, blk, :],
                            rhs=xt_s[:, c, half * QH + n * 512:
                                     half * QH + (n + 1) * 512],
                            start=(c == 0), stop=(c == ND - 1),
                        )
                _copy(eng, dst, ps)

            def emit_q_proj(pool, eng, blk, qh):
                emit_qk_proj(pool, eng, wq_s, blk, qh, qT[:, blk, qh, :])

            def emit_k_proj(pool, eng, blk, half):
                emit_qk_proj(pool, eng, wk_s, blk, half,
                             kT[:, blk, half * QH:(half + 1) * QH])

            def emit_v_proj(pool, eng, blk):
                # 4 k-chunks of V packed into one PSUM slot, one evac
                psv = pool.tile([P, 4, CDH], F32, tag="sc", name=f"pv{blk}")
                for s in range(4):
                    sc = 4 * blk + s
                    for c in range(ND):
                        nc.tensor.matmul(
                            psv[:, s, :],
                            lhsT=xt_s[:, c, sc * P:(sc + 1) * P],
                            rhs=wv_s[:, c, :],
                            start=(c == 0), stop=(c == ND - 1),
                        )
                _copy(eng, vaug[:, 4 * blk:4 * blk + 4, :, 0:DH],
                      psv.rearrange("p s (h c) -> p s h c", h=HL))

            accs = {}
            psss = {}

            def emit_scores(pools, hl, qh, kc):
                paccp, pscp, pexp, ptlp, prcp = pools
                blk, sub = hl // 2, hl % 2
                hsl = slice(64 * sub, 64 * sub + 64)
                pss = pscp.tile([P, QH], F32, tag="sc",
                                name=f"pss{hl}{qh}{kc}")
                psss[hl, qh, kc] = pss
                for n in range(2):
                    nc.tensor.matmul(
                        pss[:, n * 512:(n + 1) * 512],
                        lhsT=kT[hsl, blk, kc * P:(kc + 1) * P],
                        rhs=qT[hsl, blk, qh, n * 512:(n + 1) * 512],
                        start=True, stop=True,
                        tile_position=(64 * sub, 0),
                    )

            def emit_expav(pools, hl, qh, kc, eng):
                # exp of prefetched scores -> 8 flipped AV matmuls
                paccp, pscp, pexp, ptlp, prcp = pools
                if kc == 0:
                    acc = paccp.tile([P, NQC, P], F32, tag="acc",
                                     name=f"acc{hl}{qh}")
                    accs[hl, qh] = acc
                else:
                    acc = accs[hl, qh]
                pss = psss.pop((hl, qh, kc))
                if eng == 'A':
                    ext = pexp.tile([P, QH], F16, tag="ex", name=f"exa{kc}")
                    nc.scalar.activation(out=ext, in_=pss, func=EXP,
                                         scale=SCALE, bias=bias[:, 0:1])
                    exr = lambda sl: ext[:, sl]
                else:
                    ext = pexp.tile([P, QH], I16, tag="ex", name=f"exd{kc}")
                    nc.vector.tensor_scalar(ext, pss, A16, B16,
                                            ALU.mult, ALU.add)
                    exr = lambda sl: ext[:, sl].bitcast(F16)
                for qc in range(NQC):
                    # start=True marks a whole 2KB PSUM bank pending-zero,
                    # so only the first matmul touching each bank may carry
                    # it; later first-touch writes overwrite via the mark
                    nc.tensor.matmul(
                        acc[:, qc, 0:DH + 1],
                        lhsT=exr(slice(qc * P, (qc + 1) * P)),
                        rhs=vaug[:, kc, hl, :],
                        start=(kc == 0 and qc % 4 == 0),
                        stop=(kc == NKC - 1),
                        skip_group_check=True,
                    )

            def emit_tail(pools, hl, qh, jmin=0, jmax=NQC):
                # acc -> SBUF once (ACT), reciprocal, normalize on GPSIMD
                paccp, pscp, pexp, ptlp, prcp = pools
                if jmin == 0 and jmax == NQC:
                    acc = accs.pop((hl, qh))
                    asb = ptlp.tile([P, NQC, DH + 1], F32, tag="tl",
                                    name=f"asb{hl}{qh}")
                    _copy('A', asb, acc[:, :, 0:DH + 1])
                    rc = prcp.tile([P, NQC], F32, tag="rc",
                                   name=f"rc{hl}{qh}")
                    nc.vector.reciprocal(rc, asb[:, :, DH])
                    joff = 0
                elif jmin == 0:
                    # final unit: split tiles/engines for end latency
                    acc = accs.pop((hl, qh))
                    asb2 = ptlp.tile([P, 4, DH + 1], F32, tag="tl2",
                                     name=f"asbH{hl}{qh}")
                    _copy('D', asb2, acc[:, 4:NQC, 0:DH + 1])
                    asb = ptlp.tile([P, 4, DH + 1], F32, tag="tl",
                                    name=f"asbL{hl}{qh}")
                    _copy('D', asb, acc[:, 0:4, 0:DH + 1])
                    rc = prcp.tile([P, 4], F32, tag="rc", name=f"rcL{hl}{qh}")
                    nc.vector.reciprocal(rc, asb[:, :, DH])
                    rc2 = prcp.tile([P, 4], F32, tag="rc2",
                                    name=f"rcH{hl}{qh}")
                    nc.vector.reciprocal(rc2, asb2[:, :, DH])
                    accs[("tl", hl, qh)] = (asb2, rc2)
                    joff = 0
                else:
                    asb, rc = accs.pop(("tl", hl, qh))
                    joff = 4
                for j in range(jmin, jmax):
                    jq = qh * NQC + j
                    nc.gpsimd.tensor_scalar_mul(
                        outb[:, jq, hl * DH:(hl + 1) * DH],
                        asb[:, j - joff, 0:DH],
                        rc[:, j - joff:j - joff + 1])

            # ---------------- emission order ----------------
            out_r = out.rearrange("(j p) m -> p j m", p=P)
            with (
                tc.tile_pool(name="acc", bufs=1, space="PSUM") as paccp,
                tc.tile_pool(name="sc", bufs=3, space="PSUM") as pscp,
                tc.tile_pool(name="ex", bufs=10) as pexp,
                tc.tile_pool(name="tl", bufs=3) as ptlp,
                tc.tile_pool(name="rc", bufs=4) as prcp,
            ):
                pools = (paccp, pscp, pexp, ptlp, prcp)
                emit_k_proj(pscp, 'A', 0, 0)
                emit_k_proj(pscp, 'D', 1, 0)
                emit_q_proj(pscp, 'A', 0, 0)
                emit_q_proj(pscp, 'D', 1, 0)
                emit_v_proj(pscp, 'A', 0)

                nT = 0

                def exp_eng():
                    nonlocal nT
                    nT += 1
                    return 'AD'[(nT - 1) % 2]

                units = [(hl, qh) for qh in range(NQH) for hl in range(HL)]
                seq = [(hl, qh, kc) for (hl, qh) in units
                       for kc in range(NKC)]
                for i, (hl, qh, kc) in enumerate(seq):
                    if i == 0:
                        for w in range(3):
                            emit_scores(pools, *seq[w])
                    if i + 3 < len(seq):
                        emit_scores(pools, *seq[i + 3])
                    emit_expav(pools, hl, qh, kc, exp_eng())
                    u = i // NKC
                    # interleave remaining projections one step ahead
                    if u == 0:
                        if kc in (2, 6, 10):
                            emit_v_proj(pscp, 'A', kc // 4 + 1)
                        if kc == 0:
                            emit_k_proj(pscp, 'A', 0, 1)
                        elif kc == 4:
                            emit_k_proj(pscp, 'A', 1, 1)
                    elif u == 1 and kc == 0:
                        emit_q_proj(pscp, 'A', 0, 1)
                    elif u == 1 and kc == 8:
                        emit_q_proj(pscp, 'A', 1, 1)
                    if kc == NKC - 1:
                        if u < 7:
                            emit_tail(pools, hl, qh)
                        if u == 3:
                            # first q-half complete: stream it out
                            nc.sync.dma_start(out=out_r[:, 0:NQC, :],
                                              in_=outb[:, 0:NQC, :])
                # rows 8..15 need all four qh1 units; split the last tail
                # around the DMAs so only part of the write is exposed
                # final unit tail: A-half on ACT, then DVE half,
                # DMAs interleaved per availability
                accL = accs.pop((3, 1))
                asbL = ptlp.tile([P, 4, DH + 1], F32, tag="tl", name="asbL")
                nc.scalar.activation(out=asbL, in_=accL[:, 0:4, 0:DH + 1],
                                     func=COPY)
                rcL = prcp.tile([P, 4], F32, tag="rc", name="rcL")
                nc.vector.reciprocal(rcL, asbL[:, :, DH])
                asbH = ptlp.tile([P, 4, DH + 1], F32, tag="tl2", name="asbH")
                nc.vector.tensor_copy(asbH, accL[:, 4:NQC, 0:DH + 1])
                rcH = prcp.tile([P, 4], F32, tag="rc2", name="rcH")
                nc.vector.reciprocal(rcH, asbH[:, :, DH])
                for j in range(4):
                    nc.gpsimd.tensor_scalar_mul(
                        outb[:, NQC + j, 3 * DH:4 * DH],
                        asbL[:, j, 0:DH], rcL[:, j:j + 1])
                nc.sync.dma_start(out=out_r[:, NQC:NQC + 4, :],
                                  in_=outb[:, NQC:NQC + 4, :])
                for j in range(4, NQC):
                    nc.gpsimd.tensor_scalar_mul(
                        outb[:, NQC + j, 3 * DH:4 * DH],
                        asbH[:, j - 4, 0:DH], rcH[:, j - 4:j - 3])
                for j, q_ in zip(range(4, NQC),
                                 [nc.gpsimd, nc.scalar, nc.gpsimd,
                                  nc.scalar]):
                    q_.dma_start(out=out_r[:, NQC + j, :],
                                 in_=outb[:, NQC + j, :])

    _bass_rust.move_matmul_waits_to_ldweights(nc.m)
    _bass_rust.generate_event_semaphores(nc)
    return nc


def kernel(x, Wq, Wk, Wv):
    if "nc" not in _CACHE:
        _CACHE["nc"] = _build()
    nc = _CACHE["nc"]

    x = np.asarray(x)
    Wq, Wk, Wv = np.asarray(Wq), np.asarray(Wk), np.asarray(Wv)

    # x[b].T chunked by d: [128, 4, 2048] fp16
    xts = [np.ascontiguousarray(
        x[b].T.reshape(ND, P, S).transpose(1, 0, 2)).astype(np.float16)
        for b in range(B)]

    def pack_qk(W, hg):
        # cols within block b: (hl%2)*64 + dh; block = hl//2
        arr = W[hg * HL:(hg + 1) * HL]                # [4, 512, 64]
        arr = arr.transpose(1, 0, 2).reshape(D, 2, P)  # [d, blk, col]
        arr = arr.reshape(ND, P, 2, P)
        return np.ascontiguousarray(
            arr.transpose(1, 0, 2, 3)).astype(np.float16)

    def pack_v(W, hg):
        arr = W[hg * HL:(hg + 1) * HL]                # [4, 512, 64]
        arr = arr.transpose(1, 0, 2).reshape(D, CDH)
        arr = arr.reshape(ND, P, CDH)
        return np.ascontiguousarray(arr.transpose(1, 0, 2)).astype(np.float16)

    packs = [{"wq": pack_qk(Wq, hg), "wk": pack_qk(Wk, hg),
              "wv": pack_v(Wv, hg)} for hg in range(2)]
    in_maps = [{"xt": xts[c // 2], **packs[c % 2]} for c in range(8)]

    res = run_bass_kernel_spmd(nc, in_maps, list(range(8)))
    outf = np.empty((B, S, H * DH), np.float32)
    for c in range(8):
        b, hg = c // 2, c % 2
        outf[b, :, hg * CDH:(hg + 1) * CDH] = res.results[c]["out"]
    return outf
